# revision 49
# baseline (speedup 1.0000x reference)
"""Multi-head attention (B=4, T=2048, C=1024, H=16) on 8 trn2 NeuronCores.

Sharding: core c = 2*b + g handles batch b (of 4) and head-group g (of 2,
8 heads = 512 channels each). Each core computes q/k/v projections for its
512 channels, full TxT attention for its 8 heads, and the partial output
projection out_part = y_local @ Wo[:, g*512:(g+1)*512].T. Host sums the two
partials per batch and adds bo.

Mask trick: keys with mask!=0 contribute exactly 0 to softmax (exp(-inf)),
so the host compacts the key axis to the unmasked positions before the K/V
projections (~halves attention work). The compacted key count is padded to a
multiple of 128; padding lanes get a -1e30 bias fused into the exp.

On-chip layouts (per core):
  xT   [C=1024, T=2048]      x[b].T
  xkT  [C=1024, TKP]         compacted x[b][keep].T
  qp2  [128, 2048] x4        q.T packed: tile m holds heads 2m (part 0-63)
                             and 2m+1 (part 64-127)
  kT   [128, TKP] x4         k.T, same head packing
  vaug [TKP, 8*128]          per head 128 cols: 64 ones then 64 v data
  yT   [128, 2048] x4        normalized attention output transposed

Attention runs per head-PAIR: the two heads' score matmuls have K=64
contraction and execute on independent 64-row PE tiles (tile_position
(0,0) / (64,0)) concurrently, writing adjacent halves of one [128,1024]
psum tile that a single exp activation drains. The 64 ones-columns of
vaug replicate the softmax denominator on psum partitions 0-63, so
normalization is reciprocal+multiply straight from psum (no copy, no
partition broadcast).
"""

import numpy as np
import ml_dtypes

import concourse.bass as bass
import concourse.mybir as mybir
import concourse.tile as tile
from concourse import bacc
from concourse.bass_utils import run_bass_kernel_spmd

F32 = mybir.dt.float32
BF16 = mybir.dt.bfloat16
NP_BF16 = ml_dtypes.bfloat16

B, T, C = 4, 2048, 1024
H, D = 16, 64
G = 2                 # head groups (cores per batch)
HL = H // G           # heads per core = 8
DL = HL * D           # local channels = 512
NP = HL // 2          # head pairs per core = 4
SCALE = 1.0 / np.sqrt(D)
NEG = -1e30
N_CORES = 8

_nc_cache: dict = {}


def _dedup_ldweights(nc):
    """Drop Ldweights whose stationary operand, tile size/position and perf
    mode are identical to the immediately preceding (kept) Ldweights — the PE
    array retains its weights across matmuls, so the repeat load is pure
    overhead. Any semaphore waits on a dropped load move to the next matmul."""
    n_rm = 0
    for blk in nc.main_func.blocks:
        insts = blk.instructions
        last_key = None
        pend_waits = []
        drop = []
        for idx in range(len(insts)):
            inst = insts[idx]
            nm = type(inst).__name__
            if nm == "InstLdweights":
                a = inst.ins[0]
                key = (
                    str(getattr(a, "memref", None)),
                    str(getattr(a, "memsetref", None)),
                    a.offset, str(a.ap), str(a.dtype),
                    str(getattr(inst, "tile_size", None)),
                    str(getattr(inst, "tile_position", None)),
                    str(getattr(inst, "perf_mode", None)),
                    str(getattr(inst, "is_transpose", None)),
                )
                si = inst.sync_info
                has_upd = si is not None and len(si.on_update) > 0
                if key == last_key and not has_upd:
                    drop.append(idx)
                    if si is not None and len(si.on_wait) > 0:
                        pend_waits.extend(list(si.on_wait))
                    continue
                last_key = key
            elif nm == "InstMatmult" and pend_waits:
                si = inst.sync_info
                if si is None:
                    inst.sync_info = mybir.SyncInfo(
                        on_wait=list(pend_waits), on_update=[])
                else:
                    si.on_wait = list(si.on_wait) + list(pend_waits)
                pend_waits = []
        for idx in reversed(drop):
            del insts[idx]
        n_rm += len(drop)
    return n_rm


def _build_nc(tkp: int):
    """Build + compile the SPMD Bass program for padded key count tkp."""
    ntk = tkp // 128          # key partition-tiles
    nkc = C // 128            # contraction tiles over C = 8
    nmq = DL // 128           # channel partition-tiles = 4 (== head pairs)
    assert tkp % 128 == 0

    nc = bacc.Bacc(None, num_swdge_queues=2)

    xT_d = nc.dram_tensor("xT", [C, T], BF16, kind="ExternalInput")
    xkT_d = nc.dram_tensor("xkT", [C, tkp], BF16, kind="ExternalInput")
    wqT_d = nc.dram_tensor("wqT", [C, DL], BF16, kind="ExternalInput")
    wkT_d = nc.dram_tensor("wkT", [C, DL], BF16, kind="ExternalInput")
    wvT_d = nc.dram_tensor("wvT", [C, DL], BF16, kind="ExternalInput")
    woT_d = nc.dram_tensor("woT", [DL, C], BF16, kind="ExternalInput")
    # bias_all packs [bqp | bkp | bvp | mbp] along the free dim
    nbias = nmq + nmq + DL + ntk
    bias_d = nc.dram_tensor("bias_all", [128, nbias], F32, kind="ExternalInput")
    out_d = nc.dram_tensor("out", [T, C], mybir.dt.float16, kind="ExternalOutput")

    with tile.TileContext(nc) as tc:
        with (
            tc.tile_pool(name="persist", bufs=1) as pp,
            tc.tile_pool(name="work", bufs=4) as wp,
            tc.tile_pool(name="psum", bufs=1, space="PSUM") as psp,
        ):
            # ---- persistent SBUF tensors (wide layout: k-tile k at column k*W) ----
            def persist(shape, dt, tag):
                return pp.tile(shape, dt, tag=tag, name=tag)

            xT_a = persist([128, nkc * T], BF16, "xTa")
            xkT_a = persist([128, nkc * tkp], BF16, "xkTa")
            wqT_a = persist([128, nkc * DL], BF16, "wqTa")
            wkT_a = persist([128, nkc * DL], BF16, "wkTa")
            wvT_a = persist([128, nkc * DL], BF16, "wvTa")
            woT_a = persist([128, nmq * C], BF16, "woTa")
            qp_t = [persist([128, T], BF16, f"qp{m}") for m in range(nmq)]
            kT_t = [persist([128, tkp], BF16, f"kT{m}") for m in range(nmq)]
            va_t = [persist([128, HL * 128], BF16, f"va{t}") for t in range(ntk)]
            yT_t = [persist([128, T], BF16, f"yT{m}") for m in range(nmq)]
            bias_t = persist([128, nbias], F32, "bias")
            OQ, OK, OV, OM = 0, nmq, 2 * nmq, 2 * nmq + DL
            scr_t = persist([128, 640], BF16, "scr")  # PE warmup scratch


            # psum slots (8 banks): "s" 2x[128,1024] (4), "y" 2x[128,512] (2),
            # "f" 2x[128,512] (2)
            def psum_tile(shape, tag, name):
                return psp.tile(shape, F32, tag=tag, name=name, bufs=2)

            # ---- input DMAs: k-tile granularity, in consumption order,
            # issue fanned across four engine queues ----
            ENG = [nc.sync, nc.gpsimd]
            qi = [0]

            def dma(sb, dram, W, k0, k1, eng=None):
                src = dram[:].rearrange("(k p) n -> p k n", p=128)[:, k0:k1, :]
                dst = sb[:, k0 * W:k1 * W].rearrange("p (k n) -> p k n", n=W)
                e = ENG[qi[0] % len(ENG)] if eng is None else eng
                qi[0] += 1
                e.dma_start(out=dst, in_=src)

            nc.scalar.dma_start(out=bias_t[:], in_=bias_d[:])
            # v path first (v_units consume xkT/wvT k-tiles in order);
            # alternate queues per tile so neither queue serializes the
            # large xkT stream
            for k in range(nkc):
                dma(xkT_a, xkT_d, tkp, k, k + 1, eng=ENG[k % 2])
                dma(wvT_a, wvT_d, DL, k, k + 1, eng=ENG[(k + 1) % 2])
            # k path (k_units reuse xkT, need only wkT)
            dma(wkT_a, wkT_d, DL, 0, nkc // 2)
            dma(wkT_a, wkT_d, DL, nkc // 2, nkc)
            # q path
            for k in range(nkc):
                dma(xT_a, xT_d, T, k, k + 1)
                dma(wqT_a, wqT_d, DL, k, k + 1)
            # o weights last (first o_unit runs ~halfway through)
            dma(woT_a, woT_d, C, 0, nmq // 2)
            dma(woT_a, woT_d, C, nmq // 2, nmq)

            # ---- PE warmup: trip the HAM clock gate while DMA streams in ----
            nc.vector.memset(scr_t[:], 0.0)
            wps = psp.tile([128, 256], F32, tag="s", name="warmup", bufs=2)
            for w in range(6):
                nc.tensor.matmul(
                    wps[:], lhsT=scr_t[:, 0:128], rhs=scr_t[:, 128:384],
                    start=(w == 0), stop=(w == 5),
                )

            # va ones columns (only cols 0:64 of each head block need init)
            for t in range(ntk):
                nc.vector.memset(
                    va_t[t][:].rearrange("p (h e) -> p h e", e=128)[:, :, 0:64], 1.0)

            uid = [0]

            # ---- emission units ----
            def v_group(t_list, tags):
                # k-outer over a group of key tiles: each arriving xkT k-tile
                # feeds len(t_list) matmuls, so the startup DMA stream never
                # stalls the PE on a single tile chain
                uid[0] += 1
                pss = [psum_tile([128, DL], tg, f"vps{uid[0]}_{t}")
                       for t, tg in zip(t_list, tags)]
                for k in range(nkc):
                    for ps, t in zip(pss, t_list):
                        nc.tensor.matmul(
                            ps[:],
                            lhsT=xkT_a[:, k * tkp + t * 128:k * tkp + (t + 1) * 128],
                            rhs=wvT_a[:, k * DL:(k + 1) * DL],
                            start=(k == 0), stop=(k == nkc - 1),
                        )
                bv3 = bias_t[:, OV:OV + DL].rearrange("p (h e) -> p h e", e=D)
                for ps, t in zip(pss, t_list):
                    dst = va_t[t][:].rearrange("p (h e) -> p h e", e=128)[:, :, 64:128]
                    src = ps[:].rearrange("p (h e) -> p h e", e=D)
                    nc.vector.tensor_add(dst, src, bv3)

            def q_unit(m, ns, tag):  # ns: 512-chunk indices sharing the wq lhsT
                uid[0] += 1
                pss = [psum_tile([128, 512], tag, f"qps{uid[0]}_{n}") for n in ns]
                for k in range(nkc):
                    lhsT = wqT_a[:, k * DL + m * 128:k * DL + (m + 1) * 128]
                    for ps, n in zip(pss, ns):
                        nc.tensor.matmul(
                            ps[:], lhsT=lhsT,
                            rhs=xT_a[:, k * T + n * 512:k * T + (n + 1) * 512],
                            start=(k == 0), stop=(k == nkc - 1),
                        )
                for ps, n in zip(pss, ns):
                    nc.vector.tensor_scalar_add(
                        qp_t[m][:, n * 512:(n + 1) * 512], ps[:],
                        bias_t[:, OQ + m:OQ + m + 1]
                    )

            def k_unit(m, chunks, tag):  # chunks: [(s0, cn)] sharing wk lhsT
                uid[0] += 1
                pss = [psum_tile([128, 512], tag, f"kps{uid[0]}_{s0}")
                       for s0, cn in chunks]
                for k in range(nkc):
                    lhsT = wkT_a[:, k * DL + m * 128:k * DL + (m + 1) * 128]
                    for ps, (s0, cn) in zip(pss, chunks):
                        nc.tensor.matmul(
                            ps[:, 0:cn], lhsT=lhsT,
                            rhs=xkT_a[:, k * tkp + s0:k * tkp + s0 + cn],
                            start=(k == 0), stop=(k == nkc - 1),
                        )
                for ps, (s0, cn) in zip(pss, chunks):
                    nc.vector.tensor_scalar_add(
                        kT_t[m][:, s0:s0 + cn], ps[:, 0:cn],
                        bias_t[:, OK + m:OK + m + 1]
                    )

            def o_unit(mt, tag, tail=False):  # h2 halves share the yT lhsT per kt
                uid[0] += 1
                pss = [psum_tile([128, 512], tag, f"ops{uid[0]}_{h2}")
                       for h2 in range(2)]
                for kt in range(nmq):
                    lhsT = yT_t[kt][:, mt * 128:(mt + 1) * 128]
                    for h2 in range(2):
                        nc.tensor.matmul(
                            pss[h2][:], lhsT=lhsT,
                            rhs=woT_a[:, kt * C + h2 * 512:kt * C + (h2 + 1) * 512],
                            start=(kt == 0), stop=(kt == nmq - 1),
                        )
                for h2 in range(2):
                    o_sb = wp.tile([128, 512], mybir.dt.float16, tag="o",
                                   name=f"osb{uid[0]}_{h2}", bufs=3)
                    nc.vector.tensor_copy(o_sb[:], pss[h2][:])
                    # tail: scalar engine is idle post-attention; halve the
                    # per-dma_start issue serialization on sync
                    eng = nc.scalar if (tail and h2 == 1) else nc.sync
                    eng.dma_start(
                        out=out_d[mt * 128:(mt + 1) * 128, h2 * 512:(h2 + 1) * 512],
                        in_=o_sb[:])

            # k chunks grouped in lhsT-sharing pairs
            k_chunks = [(s0, min(512, tkp - s0)) for s0 in range(0, tkp, 512)]
            k_groups = [k_chunks[i:i + 2] for i in range(0, len(k_chunks), 2)]

            def k_units(m):
                return [lambda tag, m=m, g=g: k_unit(m, g, tag) for g in k_groups]

            # ---- startup: v proj + k(m=0) + q(0,0) only; everything else
            # (gated on the late xT DMA anyway) runs as attention fillers ----
            tags4 = ["s", "y", "s", "f"]
            for t0 in range(0, ntk, 4):
                ts = list(range(t0, min(t0 + 4, ntk)))
                v_group(ts, tags4[:len(ts)])
            for i, u in enumerate(k_units(0)):
                u(tags4[i % 4])
            q_unit(0, [0], "y")

            # filler queue consumed inside the attention loop, in need-order
            fillers = []
            for m in range(1, nmq):
                fillers += k_units(m)
                fillers.append(lambda tag, m=m: q_unit(m, [0, 1], tag))
            fillers.append(lambda tag: q_unit(0, [1, 2], tag))
            for m in range(1, nmq):
                fillers.append(lambda tag, m=m: q_unit(m, [2, 3], tag))
            fillers.append(lambda tag: q_unit(0, [3], tag))

            EXPF = mybir.ActivationFunctionType.Exp

            def attention(m, qc):
                """Head pair m (heads 2m, 2m+1), query chunk qc (512 wide)."""
                q0 = qc * 512
                uid[0] += 1
                yps = psum_tile([128, 512], "y", f"yps{uid[0]}")
                yps2 = psum_tile([128, 512], "y", f"yps2_{uid[0]}")
                pend = []  # software-pipelined PV: lag one t-step behind exp

                def pv(t, p_sb):
                    nc.tensor.matmul(
                        yps[:],
                        lhsT=va_t[t][:, (2 * m) * 128:(2 * m + 1) * 128],
                        rhs=p_sb[:, 0:512],
                        start=(t == 0), stop=(t == ntk - 1),
                    )
                    nc.tensor.matmul(
                        yps2[:],
                        lhsT=va_t[t][:, (2 * m + 1) * 128:(2 * m + 2) * 128],
                        rhs=p_sb[:, 512:1024],
                        start=(t == 0), stop=(t == ntk - 1),
                    )

                def s_mm(t):
                    # two K=64 matmuls on independent 64-row PE tiles
                    uid[0] += 1
                    s_ps = psum_tile([128, 1024], "s", f"sps{uid[0]}")
                    nc.tensor.matmul(
                        s_ps[:, 0:512],
                        lhsT=kT_t[m][0:64, t * 128:(t + 1) * 128],
                        rhs=qp_t[m][0:64, q0:q0 + 512],
                        start=True, stop=True,
                    )
                    nc.tensor.matmul(
                        s_ps[:, 512:1024],
                        lhsT=kT_t[m][64:128, t * 128:(t + 1) * 128],
                        rhs=qp_t[m][64:128, q0:q0 + 512],
                        start=True, stop=True,
                    )
                    return s_ps

                # S matmuls batched two t-steps at a time: the 64-row loads of
                # step t+1 overlap the streaming of step t's opposite tile
                for t0 in range(0, ntk, 2):
                    ts = [t for t in (t0, t0 + 1) if t < ntk]
                    sps = [s_mm(t) for t in ts]
                    for t, s_ps in zip(ts, sps):
                        p_sb = wp.tile([128, 1024], BF16, tag="p",
                                       name=f"p{uid[0]}_{t}", bufs=6)
                        nc.scalar.activation(
                            p_sb[:], s_ps[:], EXPF,
                            bias=bias_t[:, OM + t:OM + t + 1], scale=float(SCALE),
                        )
                        pend.append((t, p_sb))
                    while len(pend) > 2:
                        pv(*pend.pop(0))
                    if fillers and (FILL_EVERY[0] == 2 or t0 + 2 >= ntk):
                        fillers.pop(0)("f")
                while pend:
                    pv(*pend.pop(0))
                # normalize straight from psum: partitions 0-63 hold the
                # denominator (ones-columns), 64-127 the numerator
                uid[0] += 1
                for hp, ps in ((0, yps), (1, yps2)):
                    rec = wp.tile([128, 512], F32, tag="rec",
                                  name=f"rec{uid[0]}_{hp}", bufs=2)
                    nc.vector.reciprocal_approx_fast(rec[0:64, :], ps[0:64, :])
                    nc.vector.tensor_mul(
                        yT_t[m][64 * hp:64 * hp + 64, q0:q0 + 512],
                        ps[64:128, :], rec[0:64, :],
                    )

            FILL_EVERY = [2]
            for qc in range(T // 512):
                if qc >= 1:
                    FILL_EVERY[0] = 8
                    fillers.extend(
                        lambda tag, mt=mt: o_unit(mt, tag)
                        for mt in range(4 * (qc - 1), 4 * qc)
                    )
                for m in range(NP):
                    attention(m, qc)

            # remaining output-projection tiles; attention is done, so the
            # s/y psum banks are free — spread tags to pipeline the units
            for u in fillers:
                u("f")
            for i, mt in enumerate(range(3 * (T // 512), T // 128)):
                o_unit(mt, ["s", "y", "f", "s"][i % 4], tail=True)

    _dedup_ldweights(nc)
    nc.compile()
    return nc


def _get_nc(tkp: int):
    if tkp not in _nc_cache:
        _nc_cache[tkp] = _build_nc(tkp)
    return _nc_cache[tkp]


def kernel(x, mask, Wk, bk, Wq, bq, Wv, bv, Wo, bo, _run_kwargs=None):
    x = np.asarray(x, dtype=np.float32)
    mask = np.asarray(mask)
    Wk, bk = np.asarray(Wk, np.float32), np.asarray(bk, np.float32)
    Wq, bq = np.asarray(Wq, np.float32), np.asarray(bq, np.float32)
    Wv, bv = np.asarray(Wv, np.float32), np.asarray(bv, np.float32)
    Wo, bo = np.asarray(Wo, np.float32), np.asarray(bo, np.float32)

    keep = [np.flatnonzero(mask[b] == 0) for b in range(B)]
    max_keep = max(len(kp) for kp in keep)
    tkp = max(128, -(-max_keep // 128) * 128)
    ntk = tkp // 128
    nmq = DL // 128

    nc = _get_nc(tkp)

    in_maps = []
    for b in range(B):
        xT = np.ascontiguousarray(x[b].T).astype(NP_BF16)
        xk = np.zeros((tkp, C), np.float32)
        xk[: len(keep[b])] = x[b][keep[b]]
        xkT = np.ascontiguousarray(xk.T).astype(NP_BF16)
        mb = np.zeros(tkp, np.float32)
        mb[len(keep[b]):] = NEG
        mbp = np.ascontiguousarray(mb.reshape(ntk, 128).T)
        for g in range(G):
            gs, ge = g * DL, (g + 1) * DL
            bias_all = np.concatenate([
                bq[gs:ge].reshape(nmq, 128).T,
                bk[gs:ge].reshape(nmq, 128).T,
                np.broadcast_to(bv[gs:ge], (128, DL)),
                mbp,
            ], axis=1).astype(np.float32)
            in_maps.append({
                "xT": xT,
                "xkT": xkT,
                "wqT": np.ascontiguousarray(Wq[gs:ge].T).astype(NP_BF16),
                "wkT": np.ascontiguousarray(Wk[gs:ge].T).astype(NP_BF16),
                "wvT": np.ascontiguousarray(Wv[gs:ge].T).astype(NP_BF16),
                "woT": np.ascontiguousarray(Wo[:, gs:ge].T).astype(NP_BF16),
                "bias_all": np.ascontiguousarray(bias_all),
            })

    kw = _run_kwargs or {}
    res = run_bass_kernel_spmd(nc, in_maps, list(range(N_CORES)), **kw)

    out = np.empty((B, T, C), np.float32)
    for b in range(B):
        out[b] = (res.results[2 * b]["out"].astype(np.float32)
                  + res.results[2 * b + 1]["out"].astype(np.float32) + bo)
    if kw:
        kernel.last_result = res
    return out
